# revision 35
# baseline (speedup 1.0000x reference)
"""Axial attention (B,H,W,C)=(8,128,128,256), 8 heads, for 8 trn2 NeuronCores.

Sharding: data-parallel over batch B=8 -> one batch element per core.
Per core, two passes over x[b]:
  phase A: attention along H (one sequence per column w), writes oh to a
           bf16 HBM scratch in (H,W,C) layout.
  phase B: attention along W (one sequence per row h), adds the scratch row
           and writes the final bf16 output row (host: +bias, cast fp32).

Per-sequence math (t=128 tokens, C=256, 8 heads of e=32):
  S^T tiles DMA straight from HBM (the host pre-transposes x into
  [group, c, seq*128+t] layout).  Q/K/V projections run as fp8 DoubleRow
  matmuls (K=256 folded onto 128 partitions at 0.5 cycles/row) with
  residual compensation  x1*W1 + (x/8)*(8*(W-fp8(W))) + (x-fp8(x))*W1,
  which is 25% fewer PE cycles than bf16 2-chunk accumulation at slightly
  better precision.  V carries a fused ones-column per head so the softmax
  denominator falls out of the AV matmul; scores are computed transposed
  (keys on partitions), two heads fused per PSUM score tile so one exp
  instruction covers 1024 lanes; no max-subtraction (scores are O(1): Wq
  pre-scaled by e^-0.5 on host).  Out-projection is bf16 via PE transpose
  of the normalized attention output.

The emission is a 4-deep software pipeline over groups of G=4 sequences,
spanning BOTH phases in one 64-group loop (phase B's front overlaps phase
A's tail; the scratch read is emitted only after every overlapping write).
Each iteration emits scores(g-2) tile-interleaved with projections(g-1),
then AV(g-3) around V(g-2), then out-projection(g-4), then transposes
(g-3), so every PSUM-ring reuse (2-slot rings: scores+outproj /
projections+V / AV+transpose) waits only on work from a previous pipeline
stage and no engine round-trips within an iteration.  PSUM drains are
split DVE/Act to balance them (~5.7-6.2us per group each: Act runs exps +
V-drains, DVE runs QK-drains, normalize, transpose-drains and the phase-B
scratch add); gpsimd cannot touch PSUM so it only triggers DMAs.

Toolchain note: this neuronxcc accepts at most ONE sync-wait per
instruction; Tile's multi-wait sync is legalized post-scheduling by
hoisting extra waits onto same-engine nops.
"""

import sys

sys.path.insert(0, "/opt/trn_rl_repo")

import numpy as np
import ml_dtypes

import concourse.bass as bass
import concourse.tile as tile
from concourse import mybir
from concourse.bass_utils import run_bass_kernel_spmd
from concourse.vector_clock import ScopedClock

F32 = mybir.dt.float32
BF16 = mybir.dt.bfloat16
FP8 = mybir.dt.float8e4
PM = mybir.MatmulPerfMode
AF = mybir.ActivationFunctionType
OP = mybir.AluOpType

H = 128
W = 128
C = 256
HEADS = 8
E = C // HEADS  # 32
T = 128  # sequence length for both axes
G = 4  # sequences processed per group (batched projections)
GT = G * T

# --- workaround: this toolchain's codegen accepts at most ONE sync-wait per
# instruction; redistribute extra waits onto preceding same-engine nops. ---

_MAXW = 1


def _patched_drain_and_barrier(self, tick_clock, wait_clock):
    probe = self.nc.sync.nop(nofuse=True)
    wait_clock.add_sem_waits(probe.ins, ScopedClock({None: tick_clock.global_clock}))
    conds = list(probe.ins.sync_info.on_wait)
    probe.ins.sync_info.on_wait = conds[:_MAXW]
    rest = conds[_MAXW:]
    while rest:
        n2 = self.nc.sync.nop(nofuse=True)
        if n2.ins.sync_info is None:
            n2.ins.sync_info = mybir.SyncInfo(on_wait=[], on_update=[])
        n2.ins.sync_info.on_wait = rest[:_MAXW]
        rest = rest[_MAXW:]
    self.nc.sync.drain()
    self.nc.all_engine_barrier()
    popped = self.nc._tile_sem_poison_stack.pop()
    assert popped is self._sem_poison
    self.nc.clear_and_free_semaphores(list(self.sems.allocated().values()))
    self.nc.all_engine_barrier()


tile.TileContext._drain_and_barrier = _patched_drain_and_barrier


_CTRL_OPS = ("InstNoOp", "InstDrain", "InstEventSemaphore", "InstCompareAndBranch")


def _split_waits(nc, limit=_MAXW, compute_limit=1):
    """Hoist extra sync-waits onto fresh nops directly before their owner.

    CTRL-encoded instructions take at most one sync-wait on this toolchain;
    compute/DMA instructions take a few more.
    """
    n_split = 0
    for fn in nc.m.functions:
        for blk in fn.blocks:
            insts = blk.instructions
            out = []
            for inst in insts:
                si = inst.sync_info
                limit = (
                    _MAXW if type(inst).__name__ in _CTRL_OPS else compute_limit
                )
                if si is not None and len(si.on_wait) > limit:
                    waits = list(si.on_wait)
                    extra, keep = waits[:-limit], waits[-limit:]
                    k = 0
                    while extra:
                        nop = mybir.InstNoOp(
                            name=f"{inst.name}-wsplit{k}",
                            engine=inst.engine,
                            bass_nofuse=True,
                            sync_info=mybir.SyncInfo(
                                on_wait=extra[:limit], on_update=[]
                            ),
                        )
                        nc.register_instruction(nop, overwrite=True)
                        out.append(nop)
                        extra = extra[limit:]
                        k += 1
                        n_split += 1
                    si.on_wait = keep
                out.append(inst)
            blk.instructions = out
    return n_split


def _build():
    nc = bass.Bass("TRN2", target_bir_lowering=False, debug=False)
    KC = C // 128  # 2 contraction chunks

    # host-pre-transposed inputs: [group, c, s*128+t] so S^T tiles DMA
    # directly.  QKV projections run as fp8 DoubleRow matmuls with residual
    # compensation: x1*W1 + (x/8)*(8*(W-W1)) + (x-x1)*W1, each term one
    # DoubleRow matmul (K=256 on 128 partitions at 0.5 cycles/row), which is
    # 25% fewer PE cycles than bf16 at slightly BETTER precision.
    xt8 = {}
    for nm, ng in (("a", W // G), ("c", H // G)):
        for v in ("1", "s", "2"):
            xt8[nm, v] = nc.dram_tensor(
                f"xt{nm}{v}", [ng, C, GT], FP8, kind="ExternalInput"
            )
    w8 = {}
    for axn in (0, 1):
        for v in ("1", "s"):
            w8[axn, v] = nc.dram_tensor(
                f"w8_{axn}{v}", [128, KC, 3 * C], FP8, kind="ExternalInput"
            )
    wout0 = nc.dram_tensor("wout0", [C, C], BF16, kind="ExternalInput")
    wout1 = nc.dram_tensor("wout1", [C, C], BF16, kind="ExternalInput")
    out = nc.dram_tensor("out", [H, W, C], BF16, kind="ExternalOutput")
    scratch = nc.dram_tensor("ohs", [H, W, C], BF16)

    out_ap = out.ap()
    sc_ap = scratch.ap()
    HW4 = 4 * (E + 1)  # 132: one head-group's slice per seq

    with tile.TileContext(nc) as tc:
        with (
            tc.tile_pool(name="const", bufs=1) as const,
            tc.tile_pool(name="work", bufs=4) as work,
            tc.tile_pool(name="ohp", bufs=3) as ohp,
            tc.tile_pool(name="ps", bufs=2, space="PSUM") as ps,
            tc.tile_pool(name="pssc", bufs=2, space="PSUM") as pssc,
            tc.tile_pool(name="psproj", bufs=2, space="PSUM") as psproj,
            tc.tile_pool(name="ebp", bufs=8) as ebp,
            tc.tile_pool(name="stbp", bufs=3) as stbp,
            tc.tile_pool(name="onp", bufs=6) as onp,
        ):
            # ---- constants ----
            ident = const.tile([128, 128], BF16, tag="ident")
            from concourse.masks import make_identity

            make_identity(nc, ident)

            w8_sb = {}
            wout_sb = {}
            for ax, wout_d in enumerate([wout0, wout1]):
                wo2 = wout_d.ap().rearrange("(k p) n -> k p n", p=128)
                for v in ("1", "s"):
                    t8 = const.tile([128, KC, 3 * C], FP8, tag=f"w8{ax}{v}")
                    nc.gpsimd.dma_start(out=t8, in_=w8[ax, v].ap())
                    w8_sb[ax, v] = t8
                for k in range(KC):
                    t_o = const.tile([128, C], BF16, tag=f"wout{ax}{k}")
                    nc.gpsimd.dma_start(out=t_o, in_=wo2[k])
                    wout_sb[ax, k] = t_o

            # persistent V'-buffers (one per sequence PAIR): ones columns
            # written once, V columns overwritten every pair
            NVP = 4
            vp_bufs = []
            for i in range(NVP):
                vpb = const.tile([128, 2, HEADS * (E + 1)], BF16, tag=f"vp{i}")
                nc.gpsimd.memset(vpb, 1.0)
                vp_bufs.append(vpb)

            def full_pass(n_groups):
                """4-deep software-pipelined pass over BOTH phases.

                Global group g: phase A for g < n_groups/2 (sequences along
                H), phase B after (along W).  One pipeline spans the phase
                boundary so B's projections/scores overlap A's epilogue; the
                scratch read-after-write ordering is enforced by the tile
                framework's DRAM dependency tracking.

                iteration i emits: stb(i) DMA | scores(i-2) x projections(i-1)
                | AV(i-3) hg0 | V(i-2) | AV(i-3) hg1 | outproj(i-4) |
                transposes(i-3).
                """
                half = n_groups // 2
                st = {}  # per-group pipeline state

                def _ax(g):
                    return 0 if g < half else 1

                def stage_load(g):
                    s = st[g] = {}
                    ax = _ax(g)
                    nm = "a" if ax == 0 else "c"
                    for v in ("1", "s", "2"):
                        stb = stbp.tile(
                            [128, KC, GT], FP8, tag=f"stb{v}", name=f"stb{v}"
                        )
                        nc.sync.dma_start(
                            out=stb,
                            in_=xt8[nm, v]
                            .ap()[g % half]
                            .rearrange("(k p) t -> p k t", p=128),
                        )
                        s[f"stb{v}"] = stb

                def emit_proj(g, which, m):
                    """One QT/KT m-tile: 2 matmuls + PSUM->SBUF drain."""
                    s = st[g]
                    ax = _ax(g)
                    pp = psproj.tile([128, GT], F32, tag="psproj")
                    cols = slice(which * C + m * 128, which * C + (m + 1) * 128)
                    terms = (
                        ("1", "1"), ("s", "s"), ("1", "2"),
                    )  # (w variant, x variant)
                    for ti, (wv, xv) in enumerate(terms):
                        nc.tensor.matmul(
                            pp,
                            w8_sb[ax, wv][:, :, cols],
                            s[f"stb{xv}"],
                            start=(ti == 0),
                            stop=(ti == len(terms) - 1),
                            perf_mode=PM.DoubleRow,
                            skip_group_check=True,
                        )
                    sb = work.tile([128, GT], BF16, tag=f"qk{which}{m}")
                    if ax == 1 and which == 1 and m == KC - 1:
                        nc.scalar.activation(out=sb, in_=pp, func=AF.Copy)
                    else:
                        nc.vector.tensor_copy(out=sb, in_=pp)
                    s.setdefault("qt" if which == 0 else "kt", []).append(sb)

                def emit_scores(g, t_i):
                    """One fused score tile (2 q-heads, 2 PSUM banks)."""
                    s = st[g]
                    hg, half = divmod(t_i, 2)
                    scq2 = pssc.tile([128, 2, GT], F32, tag="pssc")
                    for qi in range(2):
                        q = half * 2 + qi
                        off = q * E
                        for sq in range(G):
                            nc.tensor.matmul(
                                scq2[:, qi, sq * T : (sq + 1) * T],
                                s["kt"][hg][off : off + E, sq * T : (sq + 1) * T],
                                s["qt"][hg][off : off + E, sq * T : (sq + 1) * T],
                                start=True,
                                stop=True,
                                tile_position=(off, 0),
                            )
                    s.setdefault("sc", []).append(scq2)

                def emit_exp(g, t_i):
                    s = st[g]
                    eb = ebp.tile([128, 2 * GT], BF16, tag="eb4")
                    nc.scalar.activation(
                        out=eb,
                        in_=s["sc"][t_i].rearrange("p a b -> p (a b)"),
                        func=AF.Exp,
                    )
                    s.setdefault("eb", []).append(eb)

                def emit_v(g):
                    """V projection + fused-ones buffer fill, per seq-pair."""
                    s = st[g]
                    ax = _ax(g)
                    s["vp"] = []
                    terms = (("1", "1"), ("s", "s"), ("2", "1"))  # (x, w)
                    for sp in range(G // 2):
                        vps = psproj.tile([128, 2, C], F32, tag="psproj")
                        for si in range(2):
                            sq = sp * 2 + si
                            for ti, (xv, wv) in enumerate(terms):
                                nc.tensor.matmul(
                                    vps[:, si],
                                    s[f"stb{xv}"][:, :, sq * T : (sq + 1) * T],
                                    w8_sb[ax, wv][:, :, 2 * C : 3 * C],
                                    start=(ti == 0),
                                    stop=(ti == len(terms) - 1),
                                    perf_mode=PM.DoubleRow,
                                    skip_group_check=True,
                                )
                        vpb = vp_bufs[(g * (G // 2) + sp) % NVP]
                        vp4 = vpb.rearrange("p s (h q) -> p s h q", q=E + 1)
                        nc.scalar.activation(
                            out=vp4[:, :, :, 0:E],
                            in_=vps.rearrange("p s (h e) -> p s h e", e=E),
                            func=AF.Copy,
                        )
                        s["vp"].append(vpb)

                def _eb(s, hg, q, sq):
                    eb = s["eb"][hg * 2 + q // 2]
                    off = (q % 2) * GT + sq * T
                    return eb[:, off : off + T]

                def emit_av(g, hg):
                    """AV + divide-normalize for one head-group (both pairs)."""
                    s = st[g]
                    if hg == 0:
                        s["onorm"] = [
                            onp.tile(
                                [128, 2 * C], BF16, tag="onorm", name=f"onorm{sp2}"
                            )
                            for sp2 in range(G // 2)
                        ]
                    for sp in range(G // 2):
                        opp = ps.tile([128, 2 * HW4], F32, tag="ps")
                        for si in range(2):
                            sq = sp * 2 + si
                            for q in range(4):
                                nc.tensor.matmul(
                                    opp[
                                        :,
                                        si * HW4
                                        + q * (E + 1) : si * HW4
                                        + (q + 1) * (E + 1),
                                    ],
                                    _eb(s, hg, q, sq),
                                    s["vp"][sp][
                                        :,
                                        si,
                                        (hg * 4 + q)
                                        * (E + 1) : (hg * 4 + q + 1)
                                        * (E + 1),
                                    ],
                                    start=True,
                                    stop=True,
                                )
                        # normalize: out = o / denom (denom lane bcast over e)
                        o4 = bass.AP(
                            tensor=opp.tensor,
                            offset=opp.offset,
                            ap=[
                                list(opp.ap[0]),
                                [HW4, 2],
                                [E + 1, 4],
                                [1, E + 1],
                            ],
                        )
                        recip = work.tile([128, 2, 4], F32, tag="recip")
                        nc.vector.reciprocal(out=recip, in_=o4[:, :, :, E])
                        ro = recip[:]
                        rb = bass.AP(
                            tensor=ro.tensor,
                            offset=ro.offset,
                            ap=[list(p) for p in ro.ap] + [[0, E]],
                        )
                        onm = s["onorm"][sp][:]
                        out_ap_n = bass.AP(
                            tensor=onm.tensor,
                            offset=onm.offset + hg * 128,
                            ap=[list(onm.ap[0]), [C, 2], [E, 4], [1, E]],
                        )
                        nc.vector.tensor_tensor(
                            out=out_ap_n,
                            in0=o4[:, :, :, 0:E],
                            in1=rb,
                            op=OP.mult,
                        )

                def emit_transp(g):
                    """onorm [q,(si,c)] -> otb [(c),(si,q)] via PE transpose."""
                    s = st[g]
                    s["otb"] = []
                    for sp in range(G // 2):
                        ot_ps = ps.tile([128, 2 * C], BF16, tag="ps")
                        for si in range(2):
                            for k in range(KC):
                                nc.tensor.transpose(
                                    ot_ps[
                                        :,
                                        si * C + k * 128 : si * C + (k + 1) * 128,
                                    ],
                                    s["onorm"][sp][
                                        :,
                                        si * C + k * 128 : si * C + (k + 1) * 128,
                                    ],
                                    ident,
                                )
                        otb = work.tile([128, 2 * C], BF16, tag="otb")
                        nc.vector.tensor_copy(out=otb, in_=ot_ps)
                        s["otb"].append(otb)

                def load_ohrow(g):
                    if _ax(g) != 1 or g not in st:
                        return
                    s = st[g]
                    ohrow = ohp.tile([128, G, C], BF16, tag="ohrow")
                    j0 = (g % half) * G
                    nc.sync.dma_start(
                        out=ohrow,
                        in_=sc_ap[j0 : j0 + G].rearrange("h w c -> w h c"),
                    )
                    s["ohrow"] = ohrow

                def emit_out(g):
                    """Out-projection, og assemble, final DMA for group g."""
                    s = st[g]
                    ax = _ax(g)
                    j0 = (g % half) * G
                    og = work.tile([128, G, C], BF16, tag="og")
                    fps_l = []
                    for sp in range(G // 2):
                        fps = pssc.tile([128, 2 * C], F32, tag="pssc")
                        for si in range(2):
                            for k in range(KC):
                                nc.tensor.matmul(
                                    fps[:, si * C : (si + 1) * C],
                                    s["otb"][sp][
                                        :,
                                        si * C + k * 128 : si * C + (k + 1) * 128,
                                    ],
                                    wout_sb[ax, k],
                                    start=(k == 0),
                                    stop=(k == KC - 1),
                                )
                        fps_l.append(fps)
                    for sp in range(G // 2):
                        fpv = fps_l[sp].rearrange("p (s c) -> p s c", c=C)
                        if ax == 0:
                            if sp == 0:
                                nc.vector.tensor_copy(
                                    out=og[:, 2 * sp : 2 * sp + 2, :], in_=fpv
                                )
                            else:
                                nc.scalar.activation(
                                    out=og[:, 2 * sp : 2 * sp + 2, :],
                                    in_=fpv,
                                    func=AF.Copy,
                                )
                        else:
                            nc.vector.tensor_tensor(
                                out=og[:, 2 * sp : 2 * sp + 2, :],
                                in0=fpv,
                                in1=s["ohrow"][:, 2 * sp : 2 * sp + 2, :],
                                op=OP.add,
                            )
                    if ax == 0:
                        nc.sync.dma_start(out=sc_ap[:, j0 : j0 + G, :], in_=og)
                    else:
                        nc.sync.dma_start(
                            out=out_ap[j0 : j0 + G].rearrange("h w c -> w h c"),
                            in_=og,
                        )
                    del st[g]

                n = n_groups
                for i in range(n + 4):
                    if i < n:
                        stage_load(i)
                    gs = i - 2  # scores group
                    gp = i - 1  # projection group
                    if 0 <= gs < n:
                        emit_scores(gs, 0)
                    if 0 <= gp < n:
                        emit_proj(gp, 0, 0)
                    if 0 <= gs < n:
                        emit_scores(gs, 1)
                        emit_exp(gs, 0)
                    if 0 <= gp < n:
                        emit_proj(gp, 0, 1)
                    if 0 <= gs < n:
                        emit_scores(gs, 2)
                        emit_exp(gs, 1)
                    if 0 <= gp < n:
                        emit_proj(gp, 1, 0)
                    if 0 <= gs < n:
                        emit_scores(gs, 3)
                        emit_exp(gs, 2)
                    if 0 <= gp < n:
                        emit_proj(gp, 1, 1)
                    if 0 <= gs < n:
                        emit_exp(gs, 3)
                    if 0 <= i - 3 < n:
                        emit_av(i - 3, 0)
                    if 0 <= gs < n:
                        emit_v(gs)
                    if 0 <= i - 3 < n:
                        emit_av(i - 3, 1)
                    if 0 <= i - 4 < n:
                        emit_out(i - 4)
                    if 0 <= i - 3 < n:
                        load_ohrow(i - 3)
                        emit_transp(i - 3)

            full_pass(2 * (W // G))

    _split_waits(nc)
    return nc


_NC = None


def _get_nc():
    global _NC
    if _NC is None:
        _NC = _build()
    return _NC


def make_in_maps(x, Wq0, Wkv0, Wout0, bout0, Wq1, Wkv1, Wout1, bout1):
    bf = ml_dtypes.bfloat16
    f8 = ml_dtypes.float8_e4m3fn
    scale = float(E) ** -0.5
    shared = {
        "wout0": np.asarray(Wout0, dtype=bf),
        "wout1": np.asarray(Wout1, dtype=bf),
    }
    # fp8 DoubleRow weights with 8x-scaled residuals, laid out [p, k, n]
    for axn, (Wq, Wkv) in enumerate(((Wq0, Wkv0), (Wq1, Wkv1))):
        wf = np.concatenate([Wq * scale, Wkv], axis=1).astype(np.float32)
        w1 = wf.astype(f8)
        w2s = ((wf - w1.astype(np.float32)) * 8.0).astype(f8)
        for v, wv in (("1", w1), ("s", w2s)):
            shared[f"w8_{axn}{v}"] = np.ascontiguousarray(
                wv.reshape(KCH, 128, 3 * C).transpose(1, 0, 2)
            )
    xf = np.asarray(x, dtype=np.float32)
    maps = []
    for b in range(x.shape[0]):
        e = xf[b]  # (H, W, C) fp32
        # xta[g, c, s*T+h] = x[h, 4g+s, c]  (phase A: sequences along H)
        xta_b = np.ascontiguousarray(
            e.transpose(1, 2, 0).reshape(W // G, G, C, H).transpose(0, 2, 1, 3)
        ).reshape(W // G, C, G * T)
        # xtc[g, c, s*T+w] = x[4g+s, w, c]  (phase B: sequences along W)
        xtc_b = np.ascontiguousarray(
            e.reshape(H // G, G, W, C).transpose(0, 3, 1, 2)
        ).reshape(H // G, C, G * T)
        m = {}
        for nm, xt in (("a", xta_b), ("c", xtc_b)):
            x1 = xt.astype(f8)
            m[f"xt{nm}1"] = x1
            m[f"xt{nm}s"] = (xt / 8.0).astype(f8)
            m[f"xt{nm}2"] = (xt - x1.astype(np.float32)).astype(f8)
        maps.append({**m, **shared})
    return maps


KCH = C // 128  # host-side contraction chunk count


def kernel(x, Wq0, Wkv0, Wout0, bout0, Wq1, Wkv1, Wout1, bout1):
    nc = _get_nc()
    in_maps = make_in_maps(
        np.asarray(x),
        np.asarray(Wq0),
        np.asarray(Wkv0),
        np.asarray(Wout0),
        np.asarray(bout0, dtype=np.float32),
        np.asarray(Wq1),
        np.asarray(Wkv1),
        np.asarray(Wout1),
        np.asarray(bout1, dtype=np.float32),
    )
    res = run_bass_kernel_spmd(nc, in_maps, core_ids=list(range(8)))
    bsum = (
        np.asarray(bout0, dtype=np.float32) + np.asarray(bout1, dtype=np.float32)
    )
    full = np.stack([np.asarray(r["out"], dtype=np.float32) for r in res.results])
    return full + bsum


# revision 36
# speedup vs baseline: 1.0048x; 1.0048x over previous
"""Axial attention (B,H,W,C)=(8,128,128,256), 8 heads, for 8 trn2 NeuronCores.

Sharding: data-parallel over batch B=8 -> one batch element per core.
Per core, two passes over x[b]:
  phase A: attention along H (one sequence per column w), writes oh to a
           bf16 HBM scratch in (H,W,C) layout.
  phase B: attention along W (one sequence per row h), adds the scratch row
           and writes the final bf16 output row (host: +bias, cast fp32).

Per-sequence math (t=128 tokens, C=256, 8 heads of e=32):
  S^T tiles DMA straight from HBM (the host pre-transposes x into
  [group, c, seq*128+t] layout).  Q/K/V projections run as fp8 DoubleRow
  matmuls (K=256 folded onto 128 partitions at 0.5 cycles/row) with
  residual compensation  x1*W1 + (x/8)*(8*(W-fp8(W))) + (x-fp8(x))*W1,
  which is 25% fewer PE cycles than bf16 2-chunk accumulation at slightly
  better precision.  V carries a fused ones-column per head so the softmax
  denominator falls out of the AV matmul; scores are computed transposed
  (keys on partitions), two heads fused per PSUM score tile so one exp
  instruction covers 1024 lanes; no max-subtraction (scores are O(1): Wq
  pre-scaled by e^-0.5 on host).  Out-projection is bf16 via PE transpose
  of the normalized attention output.

The emission is a 4-deep software pipeline over groups of G=4 sequences,
spanning BOTH phases in one 64-group loop (phase B's front overlaps phase
A's tail; the scratch read is emitted only after every overlapping write).
Each iteration emits scores(g-2) tile-interleaved with projections(g-1),
then AV(g-3) around V(g-2), then out-projection(g-4), then transposes
(g-3), so every PSUM-ring reuse (2-slot rings: scores+outproj /
projections+V / AV+transpose) waits only on work from a previous pipeline
stage and no engine round-trips within an iteration.  PSUM drains are
split DVE/Act to balance them (~5.7-6.2us per group each: Act runs exps +
V-drains, DVE runs QK-drains, normalize, transpose-drains and the phase-B
scratch add); gpsimd cannot touch PSUM so it only triggers DMAs.

Toolchain note: this neuronxcc accepts at most ONE sync-wait per
instruction; Tile's multi-wait sync is legalized post-scheduling by
hoisting extra waits onto same-engine nops.
"""

import sys

sys.path.insert(0, "/opt/trn_rl_repo")

import numpy as np
import ml_dtypes

import concourse.bass as bass
import concourse.tile as tile
from concourse import mybir
from concourse.bass_utils import run_bass_kernel_spmd
from concourse.vector_clock import ScopedClock

F32 = mybir.dt.float32
BF16 = mybir.dt.bfloat16
FP8 = mybir.dt.float8e4
PM = mybir.MatmulPerfMode
AF = mybir.ActivationFunctionType
OP = mybir.AluOpType

H = 128
W = 128
C = 256
HEADS = 8
E = C // HEADS  # 32
T = 128  # sequence length for both axes
G = 4  # sequences processed per group (batched projections)
GT = G * T

# --- workaround: this toolchain's codegen accepts at most ONE sync-wait per
# instruction; redistribute extra waits onto preceding same-engine nops. ---

_MAXW = 1


def _patched_drain_and_barrier(self, tick_clock, wait_clock):
    probe = self.nc.sync.nop(nofuse=True)
    wait_clock.add_sem_waits(probe.ins, ScopedClock({None: tick_clock.global_clock}))
    conds = list(probe.ins.sync_info.on_wait)
    probe.ins.sync_info.on_wait = conds[:_MAXW]
    rest = conds[_MAXW:]
    while rest:
        n2 = self.nc.sync.nop(nofuse=True)
        if n2.ins.sync_info is None:
            n2.ins.sync_info = mybir.SyncInfo(on_wait=[], on_update=[])
        n2.ins.sync_info.on_wait = rest[:_MAXW]
        rest = rest[_MAXW:]
    self.nc.sync.drain()
    self.nc.all_engine_barrier()
    popped = self.nc._tile_sem_poison_stack.pop()
    assert popped is self._sem_poison
    self.nc.clear_and_free_semaphores(list(self.sems.allocated().values()))
    self.nc.all_engine_barrier()


tile.TileContext._drain_and_barrier = _patched_drain_and_barrier


_CTRL_OPS = ("InstNoOp", "InstDrain", "InstEventSemaphore", "InstCompareAndBranch")


def _split_waits(nc, limit=_MAXW, compute_limit=1):
    """Hoist extra sync-waits onto fresh nops directly before their owner.

    CTRL-encoded instructions take at most one sync-wait on this toolchain;
    compute/DMA instructions take a few more.
    """
    n_split = 0
    for fn in nc.m.functions:
        for blk in fn.blocks:
            insts = blk.instructions
            out = []
            for inst in insts:
                si = inst.sync_info
                limit = (
                    _MAXW if type(inst).__name__ in _CTRL_OPS else compute_limit
                )
                if si is not None and len(si.on_wait) > limit:
                    waits = list(si.on_wait)
                    extra, keep = waits[:-limit], waits[-limit:]
                    k = 0
                    while extra:
                        nop = mybir.InstNoOp(
                            name=f"{inst.name}-wsplit{k}",
                            engine=inst.engine,
                            bass_nofuse=True,
                            sync_info=mybir.SyncInfo(
                                on_wait=extra[:limit], on_update=[]
                            ),
                        )
                        nc.register_instruction(nop, overwrite=True)
                        out.append(nop)
                        extra = extra[limit:]
                        k += 1
                        n_split += 1
                    si.on_wait = keep
                out.append(inst)
            blk.instructions = out
    return n_split


def _build():
    nc = bass.Bass("TRN2", target_bir_lowering=False, debug=False)
    KC = C // 128  # 2 contraction chunks

    # host-pre-transposed inputs: [group, c, s*128+t] so S^T tiles DMA
    # directly.  QKV projections run as fp8 DoubleRow matmuls with residual
    # compensation: x1*W1 + (x/8)*(8*(W-W1)) + (x-x1)*W1, each term one
    # DoubleRow matmul (K=256 on 128 partitions at 0.5 cycles/row), which is
    # 25% fewer PE cycles than bf16 at slightly BETTER precision.
    xt8 = {}
    for nm, ng in (("a", W // G), ("c", H // G)):
        for v in ("1", "s", "2"):
            xt8[nm, v] = nc.dram_tensor(
                f"xt{nm}{v}", [ng, C, GT], FP8, kind="ExternalInput"
            )
    w8 = {}
    for axn in (0, 1):
        for v in ("1", "s"):
            w8[axn, v] = nc.dram_tensor(
                f"w8_{axn}{v}", [128, KC, 3 * C], FP8, kind="ExternalInput"
            )
    wout0 = nc.dram_tensor("wout0", [C, C], BF16, kind="ExternalInput")
    wout1 = nc.dram_tensor("wout1", [C, C], BF16, kind="ExternalInput")
    out = nc.dram_tensor("out", [H, W, C], BF16, kind="ExternalOutput")
    scratch = nc.dram_tensor("ohs", [H, W, C], BF16)

    out_ap = out.ap()
    sc_ap = scratch.ap()
    HW4 = 4 * (E + 1)  # 132: one head-group's slice per seq

    with tile.TileContext(nc) as tc:
        with (
            tc.tile_pool(name="const", bufs=1) as const,
            tc.tile_pool(name="work", bufs=4) as work,
            tc.tile_pool(name="ohp", bufs=3) as ohp,
            tc.tile_pool(name="ps", bufs=2, space="PSUM") as ps,
            tc.tile_pool(name="pssc", bufs=2, space="PSUM") as pssc,
            tc.tile_pool(name="psproj", bufs=2, space="PSUM") as psproj,
            tc.tile_pool(name="ebp", bufs=8) as ebp,
            tc.tile_pool(name="stbp", bufs=3) as stbp,
            tc.tile_pool(name="onp", bufs=6) as onp,
        ):
            # ---- constants ----
            ident = const.tile([128, 128], BF16, tag="ident")
            from concourse.masks import make_identity

            make_identity(nc, ident)

            w8_sb = {}
            wout_sb = {}
            for ax, wout_d in enumerate([wout0, wout1]):
                wo2 = wout_d.ap().rearrange("(k p) n -> k p n", p=128)
                for v in ("1", "s"):
                    t8 = const.tile([128, KC, 3 * C], FP8, tag=f"w8{ax}{v}")
                    nc.gpsimd.dma_start(out=t8, in_=w8[ax, v].ap())
                    w8_sb[ax, v] = t8
                for k in range(KC):
                    t_o = const.tile([128, C], BF16, tag=f"wout{ax}{k}")
                    nc.gpsimd.dma_start(out=t_o, in_=wo2[k])
                    wout_sb[ax, k] = t_o

            # persistent V'-buffers (one per sequence PAIR): ones columns
            # written once, V columns overwritten every pair
            NVP = 4
            vp_bufs = []
            for i in range(NVP):
                vpb = const.tile([128, 2, HEADS * (E + 1)], BF16, tag=f"vp{i}")
                nc.gpsimd.memset(vpb, 1.0)
                vp_bufs.append(vpb)

            def full_pass(n_groups):
                """4-deep software-pipelined pass over BOTH phases.

                Global group g: phase A for g < n_groups/2 (sequences along
                H), phase B after (along W).  One pipeline spans the phase
                boundary so B's projections/scores overlap A's epilogue; the
                scratch read-after-write ordering is enforced by the tile
                framework's DRAM dependency tracking.

                iteration i emits: stb(i) DMA | scores(i-2) x projections(i-1)
                | AV(i-3) hg0 | V(i-2) | AV(i-3) hg1 | outproj(i-4) |
                transposes(i-3).
                """
                half = n_groups // 2
                st = {}  # per-group pipeline state

                def _ax(g):
                    return 0 if g < half else 1

                def stage_load(g):
                    s = st[g] = {}
                    ax = _ax(g)
                    nm = "a" if ax == 0 else "c"
                    for v in ("1", "s", "2"):
                        stb = stbp.tile(
                            [128, KC, GT], FP8, tag=f"stb{v}", name=f"stb{v}"
                        )
                        nc.sync.dma_start(
                            out=stb,
                            in_=xt8[nm, v]
                            .ap()[g % half]
                            .rearrange("(k p) t -> p k t", p=128),
                        )
                        s[f"stb{v}"] = stb

                def emit_proj(g, which, m):
                    """One QT/KT m-tile: 2 matmuls + PSUM->SBUF drain."""
                    s = st[g]
                    ax = _ax(g)
                    pp = psproj.tile([128, GT], F32, tag="psproj")
                    cols = slice(which * C + m * 128, which * C + (m + 1) * 128)
                    terms = (
                        ("1", "1"), ("s", "s"), ("1", "2"),
                    )  # (w variant, x variant)
                    for ti, (wv, xv) in enumerate(terms):
                        nc.tensor.matmul(
                            pp,
                            w8_sb[ax, wv][:, :, cols],
                            s[f"stb{xv}"],
                            start=(ti == 0),
                            stop=(ti == len(terms) - 1),
                            perf_mode=PM.DoubleRow,
                            skip_group_check=True,
                        )
                    sb = work.tile([128, GT], BF16, tag=f"qk{which}{m}")
                    if ax == 1 and which == 1 and m == KC - 1:
                        nc.scalar.activation(out=sb, in_=pp, func=AF.Copy)
                    else:
                        nc.vector.tensor_copy(out=sb, in_=pp)
                    s.setdefault("qt" if which == 0 else "kt", []).append(sb)

                def emit_scores(g, t_i):
                    """One fused score tile (2 q-heads, 2 PSUM banks)."""
                    s = st[g]
                    hg, half = divmod(t_i, 2)
                    scq2 = pssc.tile([128, 2, GT], F32, tag="pssc")
                    for qi in range(2):
                        q = half * 2 + qi
                        off = q * E
                        for sq in range(G):
                            nc.tensor.matmul(
                                scq2[:, qi, sq * T : (sq + 1) * T],
                                s["kt"][hg][off : off + E, sq * T : (sq + 1) * T],
                                s["qt"][hg][off : off + E, sq * T : (sq + 1) * T],
                                start=True,
                                stop=True,
                                tile_position=(off, 0),
                            )
                    s.setdefault("sc", []).append(scq2)

                def emit_exp(g, t_i):
                    s = st[g]
                    eb = ebp.tile([128, 2 * GT], BF16, tag="eb4")
                    nc.scalar.activation(
                        out=eb,
                        in_=s["sc"][t_i].rearrange("p a b -> p (a b)"),
                        func=AF.Exp,
                    )
                    s.setdefault("eb", []).append(eb)

                def emit_v(g):
                    """V projection + fused-ones buffer fill, per seq-pair."""
                    s = st[g]
                    ax = _ax(g)
                    s["vp"] = []
                    terms = (("1", "1"), ("s", "s"), ("2", "1"))  # (x, w)
                    for sp in range(G // 2):
                        vps = psproj.tile([128, 2, C], F32, tag="psproj")
                        for si in range(2):
                            sq = sp * 2 + si
                            for ti, (xv, wv) in enumerate(terms):
                                nc.tensor.matmul(
                                    vps[:, si],
                                    s[f"stb{xv}"][:, :, sq * T : (sq + 1) * T],
                                    w8_sb[ax, wv][:, :, 2 * C : 3 * C],
                                    start=(ti == 0),
                                    stop=(ti == len(terms) - 1),
                                    perf_mode=PM.DoubleRow,
                                    skip_group_check=True,
                                )
                        vpb = vp_bufs[(g * (G // 2) + sp) % NVP]
                        vp4 = vpb.rearrange("p s (h q) -> p s h q", q=E + 1)
                        nc.scalar.activation(
                            out=vp4[:, :, :, 0:E],
                            in_=vps.rearrange("p s (h e) -> p s h e", e=E),
                            func=AF.Copy,
                        )
                        s["vp"].append(vpb)

                def _eb(s, hg, q, sq):
                    eb = s["eb"][hg * 2 + q // 2]
                    off = (q % 2) * GT + sq * T
                    return eb[:, off : off + T]

                def emit_av(g, hg):
                    """AV + divide-normalize for one head-group (both pairs)."""
                    s = st[g]
                    if hg == 0:
                        s["onorm"] = [
                            onp.tile(
                                [128, 2 * C], BF16, tag="onorm", name=f"onorm{sp2}"
                            )
                            for sp2 in range(G // 2)
                        ]
                    for sp in range(G // 2):
                        opp = ps.tile([128, 2 * HW4], F32, tag="ps")
                        for si in range(2):
                            sq = sp * 2 + si
                            for q in range(4):
                                nc.tensor.matmul(
                                    opp[
                                        :,
                                        si * HW4
                                        + q * (E + 1) : si * HW4
                                        + (q + 1) * (E + 1),
                                    ],
                                    _eb(s, hg, q, sq),
                                    s["vp"][sp][
                                        :,
                                        si,
                                        (hg * 4 + q)
                                        * (E + 1) : (hg * 4 + q + 1)
                                        * (E + 1),
                                    ],
                                    start=True,
                                    stop=True,
                                )
                        # normalize: out = o / denom (denom lane bcast over e)
                        o4 = bass.AP(
                            tensor=opp.tensor,
                            offset=opp.offset,
                            ap=[
                                list(opp.ap[0]),
                                [HW4, 2],
                                [E + 1, 4],
                                [1, E + 1],
                            ],
                        )
                        recip = work.tile([128, 2, 4], F32, tag="recip")
                        nc.vector.reciprocal(out=recip, in_=o4[:, :, :, E])
                        ro = recip[:]
                        rb = bass.AP(
                            tensor=ro.tensor,
                            offset=ro.offset,
                            ap=[list(p) for p in ro.ap] + [[0, E]],
                        )
                        onm = s["onorm"][sp][:]
                        out_ap_n = bass.AP(
                            tensor=onm.tensor,
                            offset=onm.offset + hg * 128,
                            ap=[list(onm.ap[0]), [C, 2], [E, 4], [1, E]],
                        )
                        nc.vector.tensor_tensor(
                            out=out_ap_n,
                            in0=o4[:, :, :, 0:E],
                            in1=rb,
                            op=OP.mult,
                        )

                def emit_transp(g):
                    """onorm [q,(si,c)] -> otb [(c),(si,q)] via PE transpose."""
                    s = st[g]
                    s["otb"] = []
                    for sp in range(G // 2):
                        ot_ps = ps.tile([128, 2 * C], BF16, tag="ps")
                        for si in range(2):
                            for k in range(KC):
                                nc.tensor.transpose(
                                    ot_ps[
                                        :,
                                        si * C + k * 128 : si * C + (k + 1) * 128,
                                    ],
                                    s["onorm"][sp][
                                        :,
                                        si * C + k * 128 : si * C + (k + 1) * 128,
                                    ],
                                    ident,
                                )
                        otb = work.tile([128, 2 * C], BF16, tag="otb")
                        nc.vector.tensor_copy(out=otb, in_=ot_ps)
                        s["otb"].append(otb)

                def load_ohrow(g):
                    if _ax(g) != 1 or g not in st:
                        return
                    s = st[g]
                    ohrow = ohp.tile([128, G, C], BF16, tag="ohrow")
                    j0 = (g % half) * G
                    nc.sync.dma_start(
                        out=ohrow,
                        in_=sc_ap[j0 : j0 + G].rearrange("h w c -> w h c"),
                    )
                    s["ohrow"] = ohrow

                def emit_out(g):
                    """Out-projection, og assemble, final DMA for group g."""
                    s = st[g]
                    ax = _ax(g)
                    j0 = (g % half) * G
                    og = work.tile([128, G, C], BF16, tag="og")
                    fps_l = []
                    for sp in range(G // 2):
                        fps = pssc.tile([128, 2 * C], F32, tag="pssc")
                        for si in range(2):
                            for k in range(KC):
                                nc.tensor.matmul(
                                    fps[:, si * C : (si + 1) * C],
                                    s["otb"][sp][
                                        :,
                                        si * C + k * 128 : si * C + (k + 1) * 128,
                                    ],
                                    wout_sb[ax, k],
                                    start=(k == 0),
                                    stop=(k == KC - 1),
                                )
                        fps_l.append(fps)
                    for sp in range(G // 2):
                        fpv = fps_l[sp].rearrange("p (s c) -> p s c", c=C)
                        if ax == 0:
                            if sp == 0:
                                nc.vector.tensor_copy(
                                    out=og[:, 2 * sp : 2 * sp + 2, :], in_=fpv
                                )
                            else:
                                nc.scalar.activation(
                                    out=og[:, 2 * sp : 2 * sp + 2, :],
                                    in_=fpv,
                                    func=AF.Copy,
                                )
                        else:
                            nc.vector.tensor_tensor(
                                out=og[:, 2 * sp : 2 * sp + 2, :],
                                in0=fpv,
                                in1=s["ohrow"][:, 2 * sp : 2 * sp + 2, :],
                                op=OP.add,
                            )
                    if ax == 0:
                        nc.sync.dma_start(out=sc_ap[:, j0 : j0 + G, :], in_=og)
                    else:
                        nc.sync.dma_start(
                            out=out_ap[j0 : j0 + G].rearrange("h w c -> w h c"),
                            in_=og,
                        )
                    del st[g]

                n = n_groups
                for i in range(n + 4):
                    if i < n:
                        stage_load(i)
                    gs = i - 2  # scores group
                    gp = i - 1  # projection group
                    if 0 <= gs < n:
                        emit_scores(gs, 0)
                    if 0 <= gp < n:
                        emit_proj(gp, 0, 0)
                    if 0 <= gs < n:
                        emit_scores(gs, 1)
                        emit_exp(gs, 0)
                    if 0 <= gp < n:
                        emit_proj(gp, 0, 1)
                    if 0 <= gs < n:
                        emit_scores(gs, 2)
                        emit_exp(gs, 1)
                    if 0 <= gp < n:
                        emit_proj(gp, 1, 0)
                    if 0 <= gs < n:
                        emit_scores(gs, 3)
                        emit_exp(gs, 2)
                        emit_exp(gs, 3)
                    if 0 <= gp < n:
                        emit_proj(gp, 1, 1)
                    if 0 <= i - 3 < n:
                        emit_av(i - 3, 0)
                    if 0 <= gs < n:
                        emit_v(gs)
                    if 0 <= i - 3 < n:
                        emit_av(i - 3, 1)
                    if 0 <= i - 4 < n:
                        emit_out(i - 4)
                    if 0 <= i - 3 < n:
                        load_ohrow(i - 3)
                        emit_transp(i - 3)

            full_pass(2 * (W // G))

    _split_waits(nc)
    return nc


_NC = None


def _get_nc():
    global _NC
    if _NC is None:
        _NC = _build()
    return _NC


def make_in_maps(x, Wq0, Wkv0, Wout0, bout0, Wq1, Wkv1, Wout1, bout1):
    bf = ml_dtypes.bfloat16
    f8 = ml_dtypes.float8_e4m3fn
    scale = float(E) ** -0.5
    shared = {
        "wout0": np.asarray(Wout0, dtype=bf),
        "wout1": np.asarray(Wout1, dtype=bf),
    }
    # fp8 DoubleRow weights with 8x-scaled residuals, laid out [p, k, n]
    for axn, (Wq, Wkv) in enumerate(((Wq0, Wkv0), (Wq1, Wkv1))):
        wf = np.concatenate([Wq * scale, Wkv], axis=1).astype(np.float32)
        w1 = wf.astype(f8)
        w2s = ((wf - w1.astype(np.float32)) * 8.0).astype(f8)
        for v, wv in (("1", w1), ("s", w2s)):
            shared[f"w8_{axn}{v}"] = np.ascontiguousarray(
                wv.reshape(KCH, 128, 3 * C).transpose(1, 0, 2)
            )
    xf = np.asarray(x, dtype=np.float32)
    maps = []
    for b in range(x.shape[0]):
        e = xf[b]  # (H, W, C) fp32
        # xta[g, c, s*T+h] = x[h, 4g+s, c]  (phase A: sequences along H)
        xta_b = np.ascontiguousarray(
            e.transpose(1, 2, 0).reshape(W // G, G, C, H).transpose(0, 2, 1, 3)
        ).reshape(W // G, C, G * T)
        # xtc[g, c, s*T+w] = x[4g+s, w, c]  (phase B: sequences along W)
        xtc_b = np.ascontiguousarray(
            e.reshape(H // G, G, W, C).transpose(0, 3, 1, 2)
        ).reshape(H // G, C, G * T)
        m = {}
        for nm, xt in (("a", xta_b), ("c", xtc_b)):
            x1 = xt.astype(f8)
            m[f"xt{nm}1"] = x1
            m[f"xt{nm}s"] = (xt / 8.0).astype(f8)
            m[f"xt{nm}2"] = (xt - x1.astype(np.float32)).astype(f8)
        maps.append({**m, **shared})
    return maps


KCH = C // 128  # host-side contraction chunk count


def kernel(x, Wq0, Wkv0, Wout0, bout0, Wq1, Wkv1, Wout1, bout1):
    nc = _get_nc()
    in_maps = make_in_maps(
        np.asarray(x),
        np.asarray(Wq0),
        np.asarray(Wkv0),
        np.asarray(Wout0),
        np.asarray(bout0, dtype=np.float32),
        np.asarray(Wq1),
        np.asarray(Wkv1),
        np.asarray(Wout1),
        np.asarray(bout1, dtype=np.float32),
    )
    res = run_bass_kernel_spmd(nc, in_maps, core_ids=list(range(8)))
    bsum = (
        np.asarray(bout0, dtype=np.float32) + np.asarray(bout1, dtype=np.float32)
    )
    full = np.stack([np.asarray(r["out"], dtype=np.float32) for r in res.results])
    return full + bsum
